# revision 1
# baseline (speedup 1.0000x reference)
"""Locally-connected conv (LocalLinear) Trainium2 Bass kernel.

Problem: x (B=64, Cin=64, 32, 32), weight (Cout=64, Cin=64, 32, 32, 3, 3),
bias (Cout=64, 32, 32) -> out (B=64, Cout=64, 32, 32).
out[b,o,y,x] = sum_{c,u,v} xpad[b,c,y+u-1,x+v-1] * W[o,c,y,x,u,v] + bias[o,y,x]

Sharding: spatial rows across 8 cores (core i owns output rows y in
[4i, 4i+4) -> 128 locations/core).  Per location it's an independent
64x64 matmul with contraction 576 = Cin*9.

Per-core kernel layout (all DMA tiles use the full 128 partitions):
  - taps t=3u+v are split even/odd: even taps' weights+x live on SBUF
    partitions 0-63 (PE rows 0-63), odd taps on partitions 64-127
    (PE rows 64-127) -> row-tiled matmuls, two PSUM banks per loc-pair
    (psA even taps, psB odd taps), summed on DVE at the end.
  - locations are paired in the stationary columns: cols 0-63 = weights
    of loc (yy, 2*xp), cols 64-127 = loc (yy, 2*xp+1) -> col-tiled
    matmuls (tile_position col 0/64) sharing the PE array.
  - matmul inputs are fp16 (1 cycle/row on PE vs 4 for fp32); PSUM
    accumulates fp32; bias added in fp32; output returned fp32.
"""

import numpy as np

import concourse.bacc as bacc
import concourse.mybir as mybir
import concourse.tile as tile
from concourse.bass_utils import run_bass_kernel_spmd

NCORES = 8
B = 64
CIN = 64
COUT = 64
H = 32
ROWS_PER_CORE = H // NCORES  # 4
NJ = 64        # loc-pairs per core (4 yy rows x 16 xp)
JB = 16        # loc-pairs per weight DMA block (one yy row)
OUT_G = 8      # loc-pairs per output DMA

F16 = mybir.dt.float16
F32 = mybir.dt.float32

_nc_cache = None


def _build_nc():
    from contextlib import ExitStack

    nc = bacc.Bacc("TRN2", target_bir_lowering=False)

    w_d = nc.dram_tensor("w", [128, NJ, 5, 128], F16, kind="ExternalInput")
    xs_d = nc.dram_tensor("xs", [128, 6, 35, B], F16, kind="ExternalInput")
    b_d = nc.dram_tensor("bias_p", [128, NJ], F32, kind="ExternalInput")
    o_d = nc.dram_tensor("out_p", [128, NJ, B], F32, kind="ExternalOutput")

    with tile.TileContext(nc) as tc, ExitStack() as ctx:
        xpool = ctx.enter_context(tc.tile_pool(name="xpool", bufs=1))
        wpool = ctx.enter_context(tc.tile_pool(name="wpool", bufs=3))
        bpool = ctx.enter_context(tc.tile_pool(name="bpool", bufs=1))
        opool = ctx.enter_context(tc.tile_pool(name="opool", bufs=4))
        tpool = ctx.enter_context(tc.tile_pool(name="tpool", bufs=4))
        pspool = ctx.enter_context(tc.tile_pool(name="ps", bufs=4, space="PSUM"))

        xs_sb = xpool.tile([128, 6, 35, B], F16)
        nc.sync.dma_start(xs_sb[:], xs_d[:])
        bias_sb = bpool.tile([128, NJ], F32)
        nc.sync.dma_start(bias_sb[:], b_d[:])

        for jb in range(NJ // JB):
            w_sb = wpool.tile([128, JB, 5, 128], F16)
            nc.sync.dma_start(w_sb[:], w_d[:, jb * JB:(jb + 1) * JB, :, :])
            for g in range(JB // OUT_G):
                out_sb = opool.tile([128, OUT_G, B], F32)
                for j8 in range(OUT_G):
                    jj = g * OUT_G + j8
                    j = jb * JB + jj
                    yy, xp = divmod(j, 16)
                    xA = 2 * xp
                    xB = 2 * xp + 1
                    psA = pspool.tile([128, B], F32)
                    psB = pspool.tile([128, B], F32)
                    # Two phases (stationary cols 0-63 = loc xA, then 64-127
                    # = loc xB): the sim tracks PSUM accumulation groups per
                    # bank zero-region, so the two col-groups' accumulation
                    # groups in one bank must not interleave.
                    for xloc, tp in ((xA, 0), (xB, 64)):
                        csl = slice(tp, tp + 64)
                        for th in range(5):
                            u, v = divmod(2 * th, 3)
                            nc.tensor.matmul(
                                psA[csl, :], w_sb[0:64, jj, th, csl],
                                xs_sb[0:64, yy + u, xloc + v, :],
                                start=(th == 0), stop=(th == 4),
                                tile_position=(0, tp))
                            if th < 4:
                                u2, v2 = divmod(2 * th + 1, 3)
                                nc.tensor.matmul(
                                    psB[csl, :], w_sb[64:128, jj, th, csl],
                                    xs_sb[64:128, yy + u2, xloc + v2, :],
                                    start=(th == 0), stop=(th == 3),
                                    tile_position=(64, tp))
                    # DVE can read only one PSUM operand per op, so the
                    # drain is two DVE ops; keeping both on DVE leaves
                    # ScalarE free to issue output DMAs without queueing
                    # behind slow ACT table ops.
                    tmp = tpool.tile([128, B], F32)
                    nc.vector.tensor_scalar_add(
                        tmp[:], psB[:], bias_sb[:, j:j + 1])
                    nc.vector.tensor_add(out_sb[:, j8, :], psA[:], tmp[:])
                j0 = jb * JB + g * OUT_G
                nc.scalar.dma_start(o_d[:, j0:j0 + OUT_G, :], out_sb[:])

    nc.compile()
    return nc


def get_nc():
    global _nc_cache
    if _nc_cache is None:
        _nc_cache = _build_nc()
    return _nc_cache


def prep_inputs(x, weight, bias):
    """Host-side resharding/relayout -> list of 8 per-core input dicts."""
    x = np.asarray(x, dtype=np.float32)
    weight = np.asarray(weight, dtype=np.float32)
    bias = np.asarray(bias, dtype=np.float32)

    # x slices with halo, padded: xs[i, p, r, cx, b]
    #   p<64: c = p (even taps), p>=64: c = p-64 (odd taps), same data.
    #   local row r in [0,6) = global y 4i-1+r; window col cx = global x-1+cx
    xp_ = np.zeros((B, CIN, H + 2, H + 2), np.float32)
    xp_[:, :, 1:H + 1, 1:H + 1] = x
    xs = np.zeros((NCORES, 128, 6, 35, B), np.float16)
    for i in range(NCORES):
        s = xp_[:, :, 4 * i:4 * i + 6, :].transpose(1, 2, 3, 0)  # (c,6,34,b)
        xs[i, 0:64, :, 0:34, :] = s
        xs[i, 64:128, :, 0:34, :] = s

    # weights: wp[i, p=(pe,c), j=(yy,xp), th, oo=(xe,o)], tap t = 2*th+pe
    W10 = np.zeros((COUT, CIN, H, H, 10), np.float32)
    W10[..., :9] = weight.reshape(COUT, CIN, H, H, 9)
    A = W10.reshape(COUT, CIN, NCORES, 4, 16, 2, 5, 2)  # o c i yy xp xe th pe
    wp = A.transpose(2, 7, 1, 3, 4, 6, 5, 0).reshape(NCORES, 128, NJ, 5, 128)
    wp = np.ascontiguousarray(wp, dtype=np.float16)

    # bias: bp[i, oo=(xe,o), j=(yy,xp)]
    Bb = bias.reshape(COUT, NCORES, 4, 16, 2)  # o i yy xp xe
    bp = np.ascontiguousarray(
        Bb.transpose(1, 4, 0, 2, 3).reshape(NCORES, 128, NJ), dtype=np.float32)

    return [
        {"w": np.ascontiguousarray(wp[i]),
         "xs": np.ascontiguousarray(xs[i]),
         "bias_p": bp[i]}
        for i in range(NCORES)
    ]


def unpack_output(results):
    """results: list of 8 dicts with 'out_p' [128, NJ, B] -> (B, COUT, H, H)."""
    allout = np.stack([r["out_p"] for r in results])  # (8, 128, 64, 64)
    a = allout.reshape(NCORES, 2, COUT, 4, 16, B)     # i xe o yy xp b
    out = a.transpose(5, 2, 0, 3, 4, 1).reshape(B, COUT, H, H)
    return np.ascontiguousarray(out, dtype=np.float32)


def kernel(x, weight, bias, _trace=False, _tmpdir=None):
    nc = get_nc()
    in_maps = prep_inputs(x, weight, bias)
    res = run_bass_kernel_spmd(
        nc, in_maps, core_ids=list(range(NCORES)),
        trace=_trace, tmpdir=_tmpdir,
        **({"trace_cores": list(range(NCORES))} if _trace else {}),
    )
    out = unpack_output(res.results)
    if _trace:
        kernel.last_results = res
    return out



# revision 5
# speedup vs baseline: 1.0066x; 1.0066x over previous
"""Locally-connected conv (LocalLinear) Trainium2 Bass kernel.

Problem: x (B=64, Cin=64, 32, 32), weight (Cout=64, Cin=64, 32, 32, 3, 3),
bias (Cout=64, 32, 32) -> out (B=64, Cout=64, 32, 32).
out[b,o,y,x] = sum_{c,u,v} xpad[b,c,y+u-1,x+v-1] * W[o,c,y,x,u,v] + bias[o,y,x]

Sharding: spatial rows across 8 cores (core i owns output rows y in
[4i, 4i+4) -> 128 locations/core).  Per location it's an independent
64x64 matmul with contraction 576 = Cin*9.

Per-core layout (tap t = 3u+v):
  - xs SBUF partitions 0-63 hold x (with halo) for channel c=p; partitions
    64-127 hold the SAME x shifted one window-column left: upper[r, cx] =
    lower[r, cx+1].  Hence a K=128 matmul whose moving AP reads index
    (r, cx) contracts tap t=(u,v) on the lower half and tap t+1=(u,v+1)
    on the upper half simultaneously.  Taps pair as (0,1), (3,4), (6,7)
    -> three K=128 matmuls; taps 2, 5, 8 are K=64 matmuls placed on
    whichever half has their window (5 and 8 read the shifted upper half).
  - locations are paired in the stationary columns: two x-adjacent
    locations (xA=2*xp, xB=2*xp+1) use PE col-groups 0/64 (tile_position),
    accumulating into psum partitions 0-63 / 64-127 of one bank.
  - matmul inputs are fp16 (1 cycle/row on PE); PSUM accumulates fp32;
    bias added during the single drain op per location pair (alternating
    DVE / ACT so neither engine is the bottleneck); output stored fp16
    and upcast to fp32 on the host.
  - input DMAs are split across both HWDGE rings (sync + scalar) and
    interleaved so the first matmul's dependencies (~2.9 MB) arrive long
    before the full 13 MB input stream; outputs go out on the gpsimd
    (SWDGE) ring to keep them off the input rings.
"""

import numpy as np

import concourse.bacc as bacc
import concourse.mybir as mybir
import concourse.tile as tile
from concourse.bass_utils import run_bass_kernel_spmd

NCORES = 8
B = 64
CIN = 64
COUT = 64
H = 32
ROWS_PER_CORE = H // NCORES  # 4
NJ = 64        # loc-pairs per core (4 yy rows x 16 xp)
JB = 8         # loc-pairs per weight DMA block
WF = 576       # weight free bytes/2 per pair: 3*128 (tap pairs) + 128 (t2/t5) + 64 (t8)
OUT_G = 16     # loc-pairs per output DMA

F16 = mybir.dt.float16
F32 = mybir.dt.float32

_nc_cache = None


def _build_nc():
    from contextlib import ExitStack

    nc = bacc.Bacc("TRN2", target_bir_lowering=False)

    w_d = nc.dram_tensor("w", [128, NJ, WF], F16, kind="ExternalInput")
    xs_d = nc.dram_tensor("xs", [128, 6, 35, B], F16, kind="ExternalInput")
    b_d = nc.dram_tensor("bias_p", [128, NJ], F32, kind="ExternalInput")
    o_d = nc.dram_tensor("out_p", [128, NJ, B], F16, kind="ExternalOutput")

    with tile.TileContext(nc) as tc, ExitStack() as ctx:
        xpool = ctx.enter_context(tc.tile_pool(name="xpool", bufs=1))
        wpool = ctx.enter_context(tc.tile_pool(name="wpool", bufs=1))
        bpool = ctx.enter_context(tc.tile_pool(name="bpool", bufs=1))
        opool = ctx.enter_context(tc.tile_pool(name="opool", bufs=1))
        pspool = ctx.enter_context(tc.tile_pool(name="ps", bufs=8, space="PSUM"))

        xs_sb = xpool.tile([128, 6, 35, B], F16)
        w_sb = wpool.tile([128, NJ, WF], F16)
        bias_sb = bpool.tile([128, NJ], F32)
        out_sb = opool.tile([128, NJ, B], F16)

        # Input streaming order: two HWDGE rings run concurrently; each
        # ring is FIFO.  First matmuls need xs rows 0-2 + w block 0 (sync
        # ring) / block 1 (scalar ring).
        def xrow(eng, r):
            eng.dma_start(xs_sb[:, r], xs_d[:, r])

        def wblk(eng, b):
            eng.dma_start(w_sb[:, b * JB:(b + 1) * JB, :],
                          w_d[:, b * JB:(b + 1) * JB, :])

        nc.sync.dma_start(bias_sb[:], b_d[:])
        for r in range(6):
            xrow(nc.sync, r)
        for b in range(NJ // JB):
            wblk(nc.sync, b)

        for j in range(NJ):
            yy, xp = divmod(j, 16)
            xA = 2 * xp
            xB = xA + 1
            ps = pspool.tile([128, B], F32)
            # Two accumulation groups (psum partitions 0-63 = loc xA via
            # PE col-group 0, partitions 64-127 = loc xB via col-group 64);
            # groups in one bank must not interleave in program order.
            for g, xloc in ((0, xA), (1, xB)):
                co = 64 * g
                ksl = slice(co, co + 64)
                nc.tensor.matmul(  # taps 0+1
                    ps[ksl, :], w_sb[0:128, j, co:co + 64],
                    xs_sb[0:128, yy + 0, xloc + 0, :],
                    start=True, stop=False, tile_position=(0, co))
                nc.tensor.matmul(  # taps 3+4
                    ps[ksl, :], w_sb[0:128, j, 128 + co:128 + co + 64],
                    xs_sb[0:128, yy + 1, xloc + 0, :],
                    start=False, stop=False, tile_position=(0, co))
                nc.tensor.matmul(  # taps 6+7
                    ps[ksl, :], w_sb[0:128, j, 256 + co:256 + co + 64],
                    xs_sb[0:128, yy + 2, xloc + 0, :],
                    start=False, stop=False, tile_position=(0, co))
                nc.tensor.matmul(  # tap 2 (lower half, unshifted x)
                    ps[ksl, :], w_sb[0:64, j, 384 + co:384 + co + 64],
                    xs_sb[0:64, yy + 0, xloc + 2, :],
                    start=False, stop=False, tile_position=(0, co))
                nc.tensor.matmul(  # tap 5 (upper half, shifted x)
                    ps[ksl, :], w_sb[64:128, j, 384 + co:384 + co + 64],
                    xs_sb[64:128, yy + 1, xloc + 1, :],
                    start=False, stop=False, tile_position=(64, co))
                if g == 0:
                    nc.tensor.matmul(  # tap 8 for xA (lower half)
                        ps[ksl, :], w_sb[0:64, j, 512:576],
                        xs_sb[0:64, yy + 2, xloc + 2, :],
                        start=False, stop=True, tile_position=(0, co))
                else:
                    nc.tensor.matmul(  # tap 8 for xB (upper half)
                        ps[ksl, :], w_sb[64:128, j, 512:576],
                        xs_sb[64:128, yy + 2, xloc + 1, :],
                        start=False, stop=True, tile_position=(64, co))
            # Single drain+bias op per pair.
            nc.vector.tensor_scalar_add(
                out_sb[:, j, :], ps[:], bias_sb[:, j:j + 1])
            if j % OUT_G == OUT_G - 1:
                j0 = j - (OUT_G - 1)
                nc.scalar.dma_start(
                    o_d[:, j0:j + 1, :], out_sb[:, j0:j + 1, :])

    nc.compile()
    return nc


def get_nc():
    global _nc_cache
    if _nc_cache is None:
        _nc_cache = _build_nc()
    return _nc_cache


def prep_inputs(x, weight, bias):
    """Host-side resharding/relayout -> list of 8 per-core input dicts."""
    x = np.asarray(x, dtype=np.float32)
    weight = np.asarray(weight, dtype=np.float32)
    bias = np.asarray(bias, dtype=np.float32)

    # x with halo: xpad row slot = gy+1 (gy in -1..32), col slot = gx+1
    # (gx in -1..33; slot 34 == gx 33 is zero padding for the shifted
    # upper half).  Core i sees rows gy = 4i-1 .. 4i+4 (slots 4i..4i+5).
    xpad = np.zeros((B, CIN, H + 2, H + 3), np.float32)
    xpad[:, :, 1:H + 1, 1:H + 1] = x
    xs = np.zeros((NCORES, 128, 6, H + 3, B), np.float16)
    for i in range(NCORES):
        s = xpad[:, :, 4 * i:4 * i + 6, :].transpose(1, 2, 3, 0)  # (c,6,35,b)
        xs[i, 0:64] = s
        xs[i, 64:128, :, 0:H + 2, :] = s[:, :, 1:H + 3, :]

    # weights: w[i, p, j=(yy,xp), f]; f blocks: [t0|t1 paired, t3|t4,
    # t6|t7] at 0/128/256 (each [g=A,B ⊗ o]), [t2 lower | t5 upper] at
    # 384, [t8: A lower, B upper] at 512.
    Wr = weight.reshape(COUT, CIN, NCORES, 4, H * 9)  # o c i yy (x t)
    Wr = Wr.reshape(COUT, CIN, NCORES, 4, 16, 2, 9)   # o c i yy xp g t
    lo = Wr[..., [0, 3, 6, 2]]                        # o c i yy xp g k
    up = Wr[..., [1, 4, 7, 5]]
    # -> [i, c, yy, xp, k, g, o] -> [i, 64, 64, 512]
    wlo = lo.transpose(2, 1, 3, 4, 6, 5, 0).reshape(NCORES, CIN, NJ, 512)
    wup = up.transpose(2, 1, 3, 4, 6, 5, 0).reshape(NCORES, CIN, NJ, 512)
    w8 = Wr[..., 8]                                   # o c i yy xp g
    w8lo = w8[..., 0].transpose(1, 2, 3, 4, 0).reshape(CIN, NCORES, NJ, 64)
    w8up = w8[..., 1].transpose(1, 2, 3, 4, 0).reshape(CIN, NCORES, NJ, 64)
    wp = np.empty((NCORES, 128, NJ, WF), np.float16)
    wp[:, 0:64, :, 0:512] = wlo
    wp[:, 64:128, :, 0:512] = wup
    wp[:, 0:64, :, 512:576] = w8lo.transpose(1, 0, 2, 3)
    wp[:, 64:128, :, 512:576] = w8up.transpose(1, 0, 2, 3)

    # bias: bp[i, p=(g,o), j]
    Bb = bias.reshape(COUT, NCORES, 4, 16, 2)  # o i yy xp g
    bp = np.ascontiguousarray(
        Bb.transpose(1, 4, 0, 2, 3).reshape(NCORES, 128, NJ), dtype=np.float32)

    return [
        {"w": np.ascontiguousarray(wp[i]),
         "xs": np.ascontiguousarray(xs[i]),
         "bias_p": bp[i]}
        for i in range(NCORES)
    ]


def unpack_output(results):
    """results: list of 8 dicts with 'out_p' [128, NJ, B] -> (B, COUT, H, H)."""
    allout = np.stack([np.asarray(r["out_p"], np.float32) for r in results])
    a = allout.reshape(NCORES, 2, COUT, 4, 16, B)     # i g o yy xp b
    out = a.transpose(5, 2, 0, 3, 4, 1).reshape(B, COUT, H, H)
    return np.ascontiguousarray(out, dtype=np.float32)


def kernel(x, weight, bias, _trace=False, _tmpdir=None):
    nc = get_nc()
    in_maps = prep_inputs(x, weight, bias)
    res = run_bass_kernel_spmd(
        nc, in_maps, core_ids=list(range(NCORES)),
        trace=_trace, tmpdir=_tmpdir,
        **({"trace_cores": list(range(NCORES))} if _trace else {}),
    )
    out = unpack_output(res.results)
    if _trace:
        kernel.last_results = res
    return out


# revision 6
# speedup vs baseline: 1.0391x; 1.0323x over previous
"""Locally-connected conv (LocalLinear) Trainium2 Bass kernel.

Problem: x (B=64, Cin=64, 32, 32), weight (Cout=64, Cin=64, 32, 32, 3, 3),
bias (Cout=64, 32, 32) -> out (B=64, Cout=64, 32, 32).
out[b,o,y,x] = sum_{c,u,v} xpad[b,c,y+u-1,x+v-1] * W[o,c,y,x,u,v] + bias[o,y,x]

Sharding: spatial rows across 8 cores (core i owns output rows y in
[4i, 4i+4) -> 128 locations/core).  Per location it's an independent
64x64 matmul with contraction 576 = Cin*9.

Compute scheme (tap t = 3u+v): all matmuls are K=128 with row
tile_position 0 (HW requires a constant row position within a PSUM
accumulation group; K=128-only keeps every group uniform).
  - xs0 SBUF partitions 0-63 hold x (with halo) for channel c=p;
    partitions 64-127 hold x shifted one window-COLUMN left:
    upper[r, cx] = lower[r, cx+1].  A K=128 matmul reading index (r, cx)
    contracts tap t=(u,v) on the lower half and t+1=(u,v+1) on the upper
    half -> tap pairs (0,1), (3,4), (6,7).
  - xs1 (rows 0-3 only) holds x on partitions 0-63 and x shifted one
    window-ROW up on partitions 64-127: upper[r, cx] = lower[r+1, cx]
    -> tap pair (2,5).  Built on-chip from xs0 with SBUF->SBUF copies
    on the scalar ring (no extra HBM traffic).
  - tap 8 is a K=128 matmul whose stationary upper 64 rows are zero.
  - locations are paired in the stationary columns: two x-adjacent
    locations (xA=2*xp, xB=2*xp+1) use PE col-groups 0/64, accumulating
    into psum partitions 0-63 / 64-127 of one bank (two sequential
    accumulation groups; 5 matmuls each).
  - matmul inputs fp16; PSUM fp32; bias added in the single drain op per
    location pair (DVE); output stored fp16, upcast to fp32 on host.
  - inputs stream on the sync HWDGE ring interleaved (xs rows / weight
    blocks) so the first matmul's deps (~2.9 MB) arrive early; xs1
    copies + output DMAs ride the scalar HWDGE ring.
"""

import numpy as np

import concourse.bacc as bacc
import concourse.mybir as mybir
import concourse.tile as tile
from concourse.bass_utils import run_bass_kernel_spmd

NCORES = 8
B = 64
CIN = 64
COUT = 64
H = 32
NJ = 64        # loc-pairs per core (4 yy rows x 16 xp)
JB = 8         # loc-pairs per weight DMA block
WF = 640       # weight cols per pair: 5 blocks x 128 (g,o)
OUT_G = 16     # loc-pairs per output DMA

F16 = mybir.dt.float16
F32 = mybir.dt.float32

_nc_cache = None


def _build_nc():
    from contextlib import ExitStack

    nc = bacc.Bacc("TRN2", target_bir_lowering=False)

    w_d = nc.dram_tensor("w", [128, NJ, WF], F16, kind="ExternalInput")
    xs_d = nc.dram_tensor("xs", [128, 6, 35, B], F16, kind="ExternalInput")
    b_d = nc.dram_tensor("bias_p", [128, NJ], F32, kind="ExternalInput")
    o_d = nc.dram_tensor("out_p", [128, NJ, B], F16, kind="ExternalOutput")

    with tile.TileContext(nc) as tc, ExitStack() as ctx:
        xpool = ctx.enter_context(tc.tile_pool(name="xpool", bufs=1))
        wpool = ctx.enter_context(tc.tile_pool(name="wpool", bufs=1))
        bpool = ctx.enter_context(tc.tile_pool(name="bpool", bufs=1))
        opool = ctx.enter_context(tc.tile_pool(name="opool", bufs=1))
        pspool = ctx.enter_context(tc.tile_pool(name="ps", bufs=8, space="PSUM"))

        xs0 = xpool.tile([128, 6, 35, B], F16)
        xs1 = xpool.tile([128, 4, 35, B], F16)
        w_sb = wpool.tile([128, NJ, WF], F16)
        bias_sb = bpool.tile([128, NJ], F32)
        out_sb = opool.tile([128, NJ, B], F16)

        def xrow(r):
            nc.sync.dma_start(xs0[:, r], xs_d[:, r])

        def wblk(b):
            nc.sync.dma_start(w_sb[:, b * JB:(b + 1) * JB, :],
                              w_d[:, b * JB:(b + 1) * JB, :])

        def xs1row(r):
            # xs1 lower r <- xs0 lower r; xs1 upper r <- xs0 lower r+1
            nc.scalar.dma_start(xs1[0:64, r], xs0[0:64, r])
            nc.scalar.dma_start(xs1[64:128, r], xs0[0:64, r + 1])

        nc.sync.dma_start(bias_sb[:], b_d[:])
        xrow(0)
        xrow(1)
        xs1row(0)
        xrow(2)
        xs1row(1)
        wblk(0)
        xrow(3)
        xs1row(2)
        wblk(1)
        xrow(4)
        xs1row(3)
        wblk(2)
        xrow(5)
        for b in range(3, NJ // JB):
            wblk(b)

        for j in range(NJ):
            yy, xp = divmod(j, 16)
            ps = pspool.tile([128, B], F32)
            # col-group g: loc x = 2*xp+g -> psum partitions 64g..64g+63.
            # Each group: 5 uniform K=128 matmuls at row position 0.
            for g in (0, 1):
                xloc = 2 * xp + g
                co = 64 * g
                ksl = slice(co, co + 64)
                nc.tensor.matmul(  # taps 0+1
                    ps[ksl, :], w_sb[0:128, j, co:co + 64],
                    xs0[0:128, yy + 0, xloc + 0, :],
                    start=True, stop=False, tile_position=(0, co))
                nc.tensor.matmul(  # taps 3+4
                    ps[ksl, :], w_sb[0:128, j, 128 + co:128 + co + 64],
                    xs0[0:128, yy + 1, xloc + 0, :],
                    start=False, stop=False, tile_position=(0, co))
                nc.tensor.matmul(  # taps 6+7
                    ps[ksl, :], w_sb[0:128, j, 256 + co:256 + co + 64],
                    xs0[0:128, yy + 2, xloc + 0, :],
                    start=False, stop=False, tile_position=(0, co))
                nc.tensor.matmul(  # taps 2+5 (row-shifted variant)
                    ps[ksl, :], w_sb[0:128, j, 384 + co:384 + co + 64],
                    xs1[0:128, yy, xloc + 2, :],
                    start=False, stop=False, tile_position=(0, co))
                nc.tensor.matmul(  # tap 8 (stationary upper rows zero)
                    ps[ksl, :], w_sb[0:128, j, 512 + co:512 + co + 64],
                    xs0[0:128, yy + 2, xloc + 2, :],
                    start=False, stop=True, tile_position=(0, co))
            # Single drain+bias op per pair.
            nc.vector.tensor_scalar_add(
                out_sb[:, j, :], ps[:], bias_sb[:, j:j + 1])
            if j % OUT_G == OUT_G - 1:
                j0 = j - (OUT_G - 1)
                nc.scalar.dma_start(
                    o_d[:, j0:j + 1, :], out_sb[:, j0:j + 1, :])

    nc.compile()
    return nc


def get_nc():
    global _nc_cache
    if _nc_cache is None:
        _nc_cache = _build_nc()
    return _nc_cache


def prep_inputs(x, weight, bias):
    """Host-side resharding/relayout -> list of 8 per-core input dicts."""
    x = np.asarray(x, dtype=np.float32)
    weight = np.asarray(weight, dtype=np.float32)
    bias = np.asarray(bias, dtype=np.float32)

    # x with halo: row slot = gy+1 (gy in -1..32), col slot = gx+1
    # (gx in -1..33; slot 34 == gx 33 is zero padding for the shifted
    # upper half).  Core i sees rows gy = 4i-1 .. 4i+4 (slots 4i..4i+5).
    xpad = np.zeros((B, CIN, H + 2, H + 3), np.float32)
    xpad[:, :, 1:H + 1, 1:H + 1] = x
    xs = np.zeros((NCORES, 128, 6, H + 3, B), np.float16)
    for i in range(NCORES):
        s = xpad[:, :, 4 * i:4 * i + 6, :].transpose(1, 2, 3, 0)  # (c,6,35,b)
        xs[i, 0:64] = s
        xs[i, 64:128, :, 0:H + 2, :] = s[:, :, 1:H + 3, :]

    # weights: w[i, p, j=(yy,xp), f]; five 128-col blocks per pair
    # (f = 128k + 64g + o): k=0..2 tap pairs (0,1),(3,4),(6,7) [lower tap
    # on partitions 0-63, upper on 64-127]; k=3 taps (2,5); k=4 tap 8
    # (upper rows zero).
    Wr = weight.reshape(COUT, CIN, NCORES, 4, 16, 2, 9)  # o c i yy xp g t
    lo = Wr[..., [0, 3, 6, 2, 8]]                        # o c i yy xp g k
    up = Wr[..., [1, 4, 7, 5]]
    wlo = lo.transpose(2, 1, 3, 4, 6, 5, 0).reshape(NCORES, CIN, NJ, WF)
    wup = up.transpose(2, 1, 3, 4, 6, 5, 0).reshape(NCORES, CIN, NJ, 512)
    wp = np.zeros((NCORES, 128, NJ, WF), np.float16)
    wp[:, 0:64, :, :] = wlo
    wp[:, 64:128, :, 0:512] = wup

    # bias: bp[i, p=(g,o), j]
    Bb = bias.reshape(COUT, NCORES, 4, 16, 2)  # o i yy xp g
    bp = np.ascontiguousarray(
        Bb.transpose(1, 4, 0, 2, 3).reshape(NCORES, 128, NJ), dtype=np.float32)

    return [
        {"w": np.ascontiguousarray(wp[i]),
         "xs": np.ascontiguousarray(xs[i]),
         "bias_p": bp[i]}
        for i in range(NCORES)
    ]


def unpack_output(results):
    """results: list of 8 dicts with 'out_p' [128, NJ, B] -> (B, COUT, H, H)."""
    allout = np.stack([np.asarray(r["out_p"], np.float32) for r in results])
    a = allout.reshape(NCORES, 2, COUT, 4, 16, B)     # i g o yy xp b
    out = a.transpose(5, 2, 0, 3, 4, 1).reshape(B, COUT, H, H)
    return np.ascontiguousarray(out, dtype=np.float32)


def kernel(x, weight, bias, _trace=False, _tmpdir=None):
    nc = get_nc()
    in_maps = prep_inputs(x, weight, bias)
    res = run_bass_kernel_spmd(
        nc, in_maps, core_ids=list(range(NCORES)),
        trace=_trace, tmpdir=_tmpdir,
        **({"trace_cores": list(range(NCORES))} if _trace else {}),
    )
    out = unpack_output(res.results)
    if _trace:
        kernel.last_results = res
    return out
